# revision 14
# baseline (speedup 1.0000x reference)
"""GQA attention kernel for 8 Trainium2 NeuronCores.

Problem: B=2, S=2048, D=1024, 16 Q heads / 4 KV heads (GQA), causal,
y = softmax((x@wq+bq)(x@wk+bk)^T / 8, causal) @ (x@wv+bv) @ wo + bo

Sharding: core c -> (batch b = c//4, kv-group g = c%4). Each core computes
its batch's attention for 4 Q heads (= 1 KV head) and the partial output
projection through wo[g*256:(g+1)*256, :]. Host sums the 4 partials per
batch and adds bo.

Per-core pipeline (engineered for a continuously-busy PE so the HAM clock
gate stays at 8/8 = 2.4 GHz):
  phase 1+2 (interleaved per 512-seq group): x tiles DMA'd up front;
    PE transposes x -> xT; Q/KV projection chains run per group as soon
    as its xT columns land, overlapping the remaining x DMA.
  phase 3 attention, per (q-block, head-pair, key-tile):
    2 score matmuls -> one [128,2,512] fp32 PSUM tile; ONE wide exp on
    ACT (causally trimmed columns); prebuilt causal masks applied by DVE
    bf16 multiplies; 2 AV matmuls accumulate [65,512] (row 64 = softmax
    denominator via the ones column of vA).
    Normalization runs off the critical path: DVE evicts acc + computes
    reciprocal_approx_fast; GpSimd broadcasts 1/den across partitions;
    DVE multiplies into oT.
  phase 4 (output projection) is interleaved into the NEXT q-block's
    attention stream as PE filler; results DMA out per 128-row tile.
"""

import os
import sys
from contextlib import ExitStack

import numpy as np
import ml_dtypes

if "/opt/trn_rl_repo" not in sys.path:
    sys.path.insert(0, "/opt/trn_rl_repo")

import concourse.bass as bass
import concourse.tile as tile
from concourse import bacc, mybir
from concourse.masks import make_identity

B, S, D = 2, 2048, 1024
H, KVH, HD = 16, 4, 64
GQ = H // KVH        # 4 q heads per core
DG = GQ * HD         # 256 q dims per core
P = 128
KC = D // P          # 8 contraction chunks over D
NKT = S // P         # 16 key tiles
NQB = S // 512       # 4 query blocks
N_CORES = 8

DT = mybir.dt.float32
DTB = mybir.dt.bfloat16
AF = mybir.ActivationFunctionType
BF16 = ml_dtypes.bfloat16

_CACHE = {}


def build_nc():
    nc = bacc.Bacc(
        "TRN2",
        target_bir_lowering=False,
        debug=False,
        enable_asserts=False,
        num_devices=N_CORES,
    )
    # host pre-packs every input so each DMA is a contiguous 2D transfer
    xc = nc.dram_tensor("xc", [P, NKT, D], DTB, kind="ExternalInput").ap()
    wqd = nc.dram_tensor("wqd", [P, KC, DG], DTB, kind="ExternalInput").ap()
    wkvd = nc.dram_tensor("wkvd", [P, KC, 2 * HD], DTB, kind="ExternalInput").ap()
    wod = nc.dram_tensor("wod", [P, 2, D], DTB, kind="ExternalInput").ap()
    bqd = nc.dram_tensor("bqd", [P, 2, 1], DT, kind="ExternalInput").ap()
    bkd = nc.dram_tensor("bkd", [HD, 1], DT, kind="ExternalInput").ap()
    bvd = nc.dram_tensor("bvd", [P, 2, 1], DT, kind="ExternalInput").ap()
    out_p = nc.dram_tensor("out_p", [S, D], DT, kind="ExternalOutput").ap()

    with tile.TileContext(nc) as tc, ExitStack() as ctx:
        consts = ctx.enter_context(tc.tile_pool(name="consts", bufs=1))
        xin = ctx.enter_context(tc.tile_pool(name="xin", bufs=4))
        vtmp = ctx.enter_context(tc.tile_pool(name="vtmp", bufs=2))
        etp = ctx.enter_context(tc.tile_pool(name="etp", bufs=4))
        ysb = ctx.enter_context(tc.tile_pool(name="ysb", bufs=3))
        nrm = ctx.enter_context(tc.tile_pool(name="nrm", bufs=4))
        psS = ctx.enter_context(tc.tile_pool(name="psS", bufs=2, space="PSUM"))
        psA = ctx.enter_context(tc.tile_pool(name="psA", bufs=3, space="PSUM"))
        psY = ctx.enter_context(tc.tile_pool(name="psY", bufs=1, space="PSUM"))

        # identity built on DVE (earliest-booting engine); needed by the
        # first transpose
        ident = consts.tile([P, P], DTB, tag="ident")
        make_identity(nc, ident)

        # ---- input DMAs: few, large, contiguous; x first (PE critical path)
        xg = [consts.tile([P, 4, D], DTB, tag=f"xg{sg}", name=f"xg{sg}")
              for sg in range(4)]
        wq_all = consts.tile([P, KC, DG], DTB, tag="wq")
        wkv_all = consts.tile([P, KC, 2 * HD], DTB, tag="wkv")
        wo_all = consts.tile([P, 2, D], DTB, tag="wo")
        bq_all = consts.tile([P, 2, 1], DT, tag="bq")
        bk_sb = consts.tile([HD, 1], DT, tag="bk")
        bv_all = consts.tile([P, 2, 1], DT, tag="bv")

        nc.gpsimd.dma_start(xg[0], xc[:, 0:4, :])
        nc.sync.dma_start(xg[1], xc[:, 4:8, :])
        nc.gpsimd.dma_start(xg[2], xc[:, 8:12, :])
        nc.sync.dma_start(wq_all, wqd[:])
        nc.gpsimd.dma_start(wkv_all, wkvd[:])
        nc.gpsimd.dma_start(bk_sb, bkd[:])
        nc.sync.dma_start(xg[3], xc[:, 12:16, :])
        nc.sync.dma_start(bq_all, bqd[:])
        nc.sync.dma_start(wo_all, wod[:])
        nc.sync.dma_start(bv_all, bvd[:])

        xT = [consts.tile([P, S], DTB, tag=f"xT{dc}", name=f"xT{dc}") for dc in range(KC)]
        qT = [consts.tile([HD, S], DTB, tag=f"qT{h}", name=f"qT{h}") for h in range(GQ)]
        kT = consts.tile([HD, S], DTB, tag="kT")
        vA = consts.tile([P, NKT, HD + 1], DTB, tag="vA")
        oT = [consts.tile([P, S], DTB, tag=f"oT{c}", name=f"oT{c}") for c in range(2)]
        nc.vector.memset(vA[:, :, HD:HD + 1], 1.0)

        # causal 0/1 masks for diagonal tiles, [128, 2 heads, 512]:
        # keep iff q_local >= 128*j + p  <=>  -128j + q - p >= 0
        # (gpsimd builds them right after its DMA issues; dmask[0] is
        # needed first, at attention qb=0)
        dmask = []
        for j in range(4):
            mt = consts.tile([P, 2, 512], DTB, tag=f"dmask{j}", name=f"dmask{j}")
            nc.gpsimd.memset(mt, 1.0)
            nc.gpsimd.affine_select(
                out=mt, in_=mt, pattern=[[0, 2], [1, 512]],
                compare_op=mybir.AluOpType.is_ge, fill=0.0,
                base=-128 * j, channel_multiplier=-1)
            dmask.append(mt)

        filler = []      # (st, nb2) output-projection jobs ready to emit
        tl = [0]

        def emit_filler(tail=False):
            st, nb2 = filler.pop(0)
            tl[0] += 1
            # during the drain (attention pools idle) rotate psums through
            # psS as well so jobs pipeline 3-deep instead of serializing on
            # the single psY bank
            if tail and tl[0] % 3:
                yps = psS.tile([P, 512], DT, tag="sd", name=f"yps{tl[0]}")
            else:
                yps = psY.tile([P, 512], DT, tag="psy", name=f"ypsy{tl[0]}")
            for c in range(2):
                nc.tensor.matmul(
                    yps, oT[c][:, st * P:(st + 1) * P],
                    wo_all[:, c, nb2 * 512:(nb2 + 1) * 512],
                    start=(c == 0), stop=(c == 1))
            yt = ysb.tile([P, 512], DT, tag="y")
            if tail and tl[0] % 2:
                nc.scalar.activation(yt, yps, AF.Identity)
            else:
                nc.vector.tensor_copy(yt, yps)
            (nc.sync if (st * 2 + nb2) % 2 else nc.gpsimd).dma_start(
                out_p[st * P:(st + 1) * P, nb2 * 512:(nb2 + 1) * 512], yt)

        # ---- fully fused pipeline, per 512-seq group sg:
        #   transpose x_sg -> xT cols; project Q/KV for sg;
        #   attention q-block qb=sg (+ phase-4 filler of qb-1)
        cp_i = 0
        for sg in range(4):
            sl = slice(sg * 512, (sg + 1) * 512)
            for dc in range(KC):
                ps = psA.tile([P, 512], DTB, tag="ps")
                for j in range(4):
                    nc.tensor.transpose(
                        ps[:, j * P:(j + 1) * P],
                        xg[sg][:, j, dc * P:(dc + 1) * P], ident)
                # alternate PSUM->SBUF copies between DVE and ACT
                cp_i += 1
                if (cp_i % 8) < 5:
                    nc.vector.tensor_copy(xT[dc][:, sl], ps)
                else:
                    nc.scalar.activation(xT[dc][:, sl], ps, AF.Identity)
            # Q projection for this seq group
            for mc in range(2):
                ps = psA.tile([P, 512], DT, tag="ps")
                for kc in range(KC):
                    nc.tensor.matmul(
                        ps, wq_all[:, kc, mc * P:(mc + 1) * P], xT[kc][:, sl],
                        start=(kc == 0), stop=(kc == KC - 1))
                for hh in range(2):
                    h = mc * 2 + hh
                    nc.scalar.activation(
                        qT[h][:, sl], ps[hh * HD:(hh + 1) * HD, :], AF.Identity,
                        bias=bq_all[hh * HD:(hh + 1) * HD, mc, :], scale=0.125)
            # KV projection for this seq group
            ps2 = psA.tile([P, 512], DT, tag="ps")
            for kc in range(KC):
                nc.tensor.matmul(
                    ps2, wkv_all[:, kc, :], xT[kc][:, sl],
                    start=(kc == 0), stop=(kc == KC - 1))
            nc.scalar.activation(kT[:, sl], ps2[0:HD, :], AF.Identity, bias=bk_sb)
            vt = vtmp.tile([HD, 512], DTB, tag="vtmp")
            nc.vector.tensor_copy(vt, ps2[HD:2 * HD, :])
            for j in range(4):
                kt = sg * 4 + j
                vps = psA.tile([P, HD], DTB, tag="ps")
                nc.tensor.transpose(vps, vt[:, j * P:(j + 1) * P], ident[0:HD, 0:HD])
                nc.vector.tensor_copy(vA[:, kt, 0:HD], vps)

            # ---- attention q-block qb = sg ----
            qb = sg
            qsl = sl
            nkt = 4 * (qb + 1)
            n_iters = 2 * nkt
            gap = max(1, n_iters // len(filler)) if filler else 0
            it = 0
            for pp in range(2):          # head-pair pass
                accs = [
                    psA.tile([HD + 1, 512], DT, tag="ps", name=f"acc{qb}_{pp}_{i}")
                    for i in range(2)
                ]
                for kt in range(nkt):
                    sps = psS.tile([P, 2, 512], DT, tag="sd")
                    for hh in range(2):
                        h = pp * 2 + hh
                        nc.tensor.matmul(
                            sps[:, hh, :], kT[:, kt * P:(kt + 1) * P],
                            qT[h][:, qsl], start=True, stop=True)
                    j = kt - 4 * qb
                    # columns < 128*j are fully masked: skip them end-to-end
                    # (exp, mask, AV). kt==0 always has c0==0, so the psum
                    # has_written bits cover the full accumulator width.
                    c0 = max(j, 0) * P
                    et = etp.tile([P, 2, 512], DTB, tag="et")
                    nc.scalar.activation(
                        et[:, :, c0:], sps[:, :, c0:], AF.Exp)
                    if j >= 0:
                        nc.vector.tensor_mul(
                            et[:, :, c0:], et[:, :, c0:], dmask[j][:, :, c0:])
                    for hh in range(2):
                        nc.tensor.matmul(
                            accs[hh][:, c0:], vA[:, kt, :], et[:, hh, c0:],
                            start=(kt == 0), stop=(kt == nkt - 1))
                    it += 1
                    if filler and gap and it % gap == 0:
                        emit_filler()
                # normalize this pass's two heads (off PE critical path)
                for hh in range(2):
                    h = pp * 2 + hh
                    c, r0 = h // 2, (h % 2) * HD
                    oU = nrm.tile([HD, 512], DTB, tag="oU")
                    nc.vector.tensor_copy(oU, accs[hh][0:HD, :])
                    dsb = nrm.tile([1, 512], DT, tag="dsb")
                    nc.vector.tensor_copy(dsb, accs[hh][HD:HD + 1, :])
                    rcp = nrm.tile([1, 512], DT, tag="rcp")
                    # custom-DVE op: SBUF operands only (PSUM input returns
                    # garbage on HW; verified empirically)
                    nc.vector.reciprocal_approx_fast(out=rcp, in_=dsb)
                    rcpb = nrm.tile([1, 512], DTB, tag="rcpb")
                    nc.vector.tensor_copy(rcpb, rcp)
                    rbb = nrm.tile([HD, 512], DTB, tag="rbb")
                    nc.gpsimd.partition_broadcast(rbb, rcpb, channels=HD)
                    nc.vector.tensor_mul(oT[c][r0:r0 + HD, qsl], oU, rbb)
                # v bias (equivalent to adding bv to v since sum(attn)=1)
                nc.vector.tensor_scalar_add(
                    oT[pp][:, qsl], oT[pp][:, qsl], bv_all[:, pp, :])
            while filler:
                emit_filler(tail=(qb == NQB - 1))
            for st in range(4 * qb, 4 * qb + 4):
                filler.append((st, 0))
                filler.append((st, 1))
        while filler:
            emit_filler(tail=True)

    nc.compile()
    return nc


def kernel(x, mask, wq, bq, wk, bk, wv, bv, wo, bo):
    x = np.asarray(x, dtype=np.float32)
    wq = np.asarray(wq, dtype=np.float32)
    wk = np.asarray(wk, dtype=np.float32)
    wv = np.asarray(wv, dtype=np.float32)
    wo = np.asarray(wo, dtype=np.float32)
    bq = np.asarray(bq, dtype=np.float32)
    bk = np.asarray(bk, dtype=np.float32)
    bv = np.asarray(bv, dtype=np.float32)
    bo = np.asarray(bo, dtype=np.float32)

    xb = x.astype(BF16)
    wqb = wq.astype(BF16)
    wkb = wk.astype(BF16)
    wvb = wv.astype(BF16)
    wob = wo.astype(BF16)

    def pack(a, groups):
        # [groups*128, F] -> [128, groups, F]
        return np.ascontiguousarray(
            a.reshape(groups, P, -1).transpose(1, 0, 2))

    in_maps = []
    for c in range(N_CORES):
        b, g = c // 4, c % 4
        sq = slice(g * DG, (g + 1) * DG)
        sk = slice(g * HD, (g + 1) * HD)
        wkv = np.concatenate([wkb[:, sk], wvb[:, sk]], axis=1)  # [D, 128]
        in_maps.append({
            "xc": pack(xb[b], NKT),
            "wqd": pack(wqb[:, sq], KC),
            "wkvd": pack(wkv, KC),
            "wod": pack(wob[sq, :], 2),
            "bqd": pack((bq[sq] * 0.125).reshape(DG, 1), 2),
            "bkd": np.ascontiguousarray(bk[sk].reshape(HD, 1)),
            "bvd": pack(np.tile(bv[sk], GQ).reshape(DG, 1), 2),
        })

    results = _run(in_maps)

    out = np.empty((B, S, D), dtype=np.float32)
    for b in range(B):
        acc = results[b * 4 + 0]["out_p"].astype(np.float64)
        for g in range(1, 4):
            acc += results[b * 4 + g]["out_p"]
        out[b] = (acc + bo).astype(np.float32)
    return out


def _get_runner():
    """Build (once) a jitted shard_map callable executing the compiled
    kernel on 8 cores. Adapted from concourse.bass2jax.run_bass_via_pjrt,
    minus output-buffer donation so the callable is re-invokable for
    timing."""
    if "runner" in _CACHE:
        return _CACHE["runner"]
    import jax
    from jax.experimental.shard_map import shard_map
    from jax.sharding import Mesh, PartitionSpec
    from concourse import bass2jax
    from concourse.bass2jax import _bass_exec_p, install_neuronx_cc_hook

    install_neuronx_cc_hook()
    nc = build_nc()
    _CACHE["nc"] = nc
    partition_name = (
        nc.partition_id_tensor.name if nc.partition_id_tensor else None
    )

    in_names, out_names, out_avals, zero_outs = [], [], [], []
    for alloc in nc.m.functions[0].allocations:
        if not isinstance(alloc, mybir.MemoryLocationSet):
            continue
        name = alloc.memorylocations[0].name
        if alloc.kind == "ExternalInput":
            if name != partition_name:
                in_names.append(name)
        elif alloc.kind == "ExternalOutput":
            out_names.append(name)
            shape = tuple(alloc.tensor_shape)
            dtype = mybir.dt.np(alloc.dtype)
            out_avals.append(jax.core.ShapedArray(shape, dtype))
            zero_outs.append(np.zeros(shape, dtype))
    n_params = len(in_names)
    all_names = in_names + out_names
    if partition_name is not None:
        all_names = all_names + [partition_name]

    def _body(*args):
        operands = list(args)
        if partition_name is not None:
            operands.append(bass2jax.partition_id_tensor())
        outs = _bass_exec_p.bind(
            *operands,
            out_avals=tuple(out_avals),
            in_names=tuple(all_names),
            out_names=tuple(out_names),
            lowering_input_output_aliases=(),
            sim_require_finite=True,
            sim_require_nnan=True,
            nc=nc,
        )
        return tuple(outs)

    devices = jax.devices()[:N_CORES]
    mesh = Mesh(np.asarray(devices), ("core",))
    n_all = n_params + len(out_names)
    sharded = jax.jit(
        shard_map(
            _body,
            mesh=mesh,
            in_specs=(PartitionSpec("core"),) * n_all,
            out_specs=(PartitionSpec("core"),) * len(out_names),
            check_rep=False,
        ),
        keep_unused=True,
    )
    runner = {
        "sharded": sharded,
        "in_names": in_names,
        "out_names": out_names,
        "out_avals": out_avals,
        "zero_outs": zero_outs,
        "mesh": mesh,
    }
    _CACHE["runner"] = runner
    return runner


def _run(in_maps):
    r = _get_runner()
    concat_in = [
        np.concatenate([np.asarray(in_maps[c][n]) for c in range(N_CORES)], axis=0)
        for n in r["in_names"]
    ]
    concat_zeros = [
        np.zeros((N_CORES * z.shape[0], *z.shape[1:]), z.dtype)
        for z in r["zero_outs"]
    ]
    out_arrs = r["sharded"](*concat_in, *concat_zeros)
    _CACHE["last_args"] = (concat_in, concat_zeros)
    return [
        {
            n: np.asarray(out_arrs[i]).reshape(
                N_CORES, *r["out_avals"][i].shape
            )[c]
            for i, n in enumerate(r["out_names"])
        }
        for c in range(N_CORES)
    ]


def bench(iters=10):
    """Re-execute the last-run kernel with device-resident inputs and
    return per-call wall times (s). Outputs stay on device."""
    import time as _time
    import jax
    from jax.sharding import NamedSharding, PartitionSpec

    r = _CACHE["runner"]
    concat_in, concat_zeros = _CACHE["last_args"]
    sh = NamedSharding(r["mesh"], PartitionSpec("core"))
    dev_args = [jax.device_put(a, sh) for a in (*concat_in, *concat_zeros)]
    for a in dev_args:
        a.block_until_ready()
    times = []
    for _ in range(iters):
        t0 = _time.perf_counter()
        outs = r["sharded"](*dev_args)
        for o in outs:
            o.block_until_ready()
        times.append(_time.perf_counter() - t0)
    return times


def profile_exec(prof_dir=None):
    """Capture an NTFF profile (neuron-profile) of one device-resident
    re-execution; return (max_exec_time_ns, per_core_ns, trace_paths)."""
    import contextlib
    import ctypes
    import tempfile
    import jax
    from jax.sharding import NamedSharding, PartitionSpec

    r = _CACHE["runner"]
    concat_in, concat_zeros = _CACHE["last_args"]
    sh = NamedSharding(r["mesh"], PartitionSpec("core"))
    dev_args = [jax.device_put(a, sh) for a in (*concat_in, *concat_zeros)]
    for a in dev_args:
        a.block_until_ready()
    # warm run
    outs = r["sharded"](*dev_args)
    for o in outs:
        o.block_until_ready()

    lib = ctypes.CDLL("/opt/axon/libaxon_pjrt.so")
    lib.axon_start_nrt_profile.argtypes = [
        ctypes.POINTER(ctypes.c_int64), ctypes.c_size_t]
    lib.axon_start_nrt_profile.restype = ctypes.c_int64
    lib.axon_stop_nrt_profile.argtypes = [ctypes.c_char_p]
    lib.axon_stop_nrt_profile.restype = ctypes.c_int64

    if prof_dir is None:
        prof_dir = tempfile.mkdtemp(prefix="ntffprof_")
    ids = (ctypes.c_int64 * N_CORES)(*range(N_CORES))
    rc = lib.axon_start_nrt_profile(ids, N_CORES)
    if rc != 0:
        raise RuntimeError(f"axon_start_nrt_profile rc={rc}")
    try:
        outs = r["sharded"](*dev_args)
        for o in outs:
            o.block_until_ready()
    finally:
        n = lib.axon_stop_nrt_profile(str(prof_dir).encode())
        if n <= 0:
            raise RuntimeError(f"axon_stop_nrt_profile wrote {n} files")

    import gauge.profiler
    profile = gauge.profiler.Profile(
        profile_path=gauge.profiler.FishPath(prof_dir),
        kernel_dev_mode=True,
        profile_on_exit=False,
        bass_kernel=_CACHE["nc"].m,
        offline_processing=True,
        fname="*_body*",
    )
    ntffs = profile.find_ntffs()
    model_indices = tuple(sorted(set(n.model_index for n in ntffs)))
    results = profile.to_perfetto(model_index=model_indices)
    per_core = {mi: res.exec_time_ns for mi, res in zip(model_indices, results)}
    traces = {mi: res.trace_path for mi, res in zip(model_indices, results)}
    return max(per_core.values()), per_core, traces, prof_dir


# revision 16
# speedup vs baseline: 1.1587x; 1.1587x over previous
"""GQA attention kernel for 8 Trainium2 NeuronCores.

Problem: B=2, S=2048, D=1024, 16 Q heads / 4 KV heads (GQA), causal,
y = softmax((x@wq+bq)(x@wk+bk)^T / 8, causal) @ (x@wv+bv) @ wo + bo

Sharding: core c -> (batch b = c//4, kv-group g = c%4). Each core computes
its batch's attention for 4 Q heads (= 1 KV head) and the partial output
projection through wo[g*256:(g+1)*256, :]. Host sums the 4 partials per
batch and adds bo.

Per-core pipeline (engineered for a continuously-busy PE so the HAM clock
gate stays at 8/8 = 2.4 GHz):
  phase 1+2 (interleaved per 512-seq group): x tiles DMA'd up front;
    PE transposes x -> xT; Q/KV projection chains run per group as soon
    as its xT columns land, overlapping the remaining x DMA.
  phase 3 attention, per (q-block, head-pair, key-tile):
    2 score matmuls -> one [128,2,512] fp32 PSUM tile; ONE wide exp on
    ACT (causally trimmed columns); prebuilt causal masks applied by DVE
    bf16 multiplies; 2 AV matmuls accumulate [65,512] (row 64 = softmax
    denominator via the ones column of vA).
    Normalization runs off the critical path: DVE evicts acc + computes
    reciprocal_approx_fast; GpSimd broadcasts 1/den across partitions;
    DVE multiplies into oT.
  phase 4 (output projection) is interleaved into the NEXT q-block's
    attention stream as PE filler; results DMA out per 128-row tile.
"""

import os
import sys
from contextlib import ExitStack

import numpy as np
import ml_dtypes

if "/opt/trn_rl_repo" not in sys.path:
    sys.path.insert(0, "/opt/trn_rl_repo")

import concourse.bass as bass
import concourse.tile as tile
from concourse import bacc, mybir
from concourse.masks import make_identity

B, S, D = 2, 2048, 1024
H, KVH, HD = 16, 4, 64
GQ = H // KVH        # 4 q heads per core
DG = GQ * HD         # 256 q dims per core
P = 128
KC = D // P          # 8 contraction chunks over D
NKT = S // P         # 16 key tiles
NQB = S // 512       # 4 query blocks
N_CORES = 8

DT = mybir.dt.float32
DTB = mybir.dt.bfloat16
AF = mybir.ActivationFunctionType
BF16 = ml_dtypes.bfloat16

_CACHE = {}


def build_nc():
    nc = bacc.Bacc(
        "TRN2",
        target_bir_lowering=False,
        debug=False,
        enable_asserts=False,
        num_devices=N_CORES,
    )
    # host pre-packs every input so each DMA is a contiguous 2D transfer
    xc = nc.dram_tensor("xc", [P, NKT, D], DTB, kind="ExternalInput").ap()
    wqd = nc.dram_tensor("wqd", [P, KC, DG], DTB, kind="ExternalInput").ap()
    wkvd = nc.dram_tensor("wkvd", [P, KC, 2 * HD], DTB, kind="ExternalInput").ap()
    wod = nc.dram_tensor("wod", [P, 2, D], DTB, kind="ExternalInput").ap()
    bqd = nc.dram_tensor("bqd", [P, 2, 1], DT, kind="ExternalInput").ap()
    bkd = nc.dram_tensor("bkd", [HD, 1], DT, kind="ExternalInput").ap()
    bvd = nc.dram_tensor("bvd", [P, 2, 1], DT, kind="ExternalInput").ap()
    out_p = nc.dram_tensor("out_p", [S, D], DT, kind="ExternalOutput").ap()

    with tile.TileContext(nc) as tc, ExitStack() as ctx:
        consts = ctx.enter_context(tc.tile_pool(name="consts", bufs=1))
        xin = ctx.enter_context(tc.tile_pool(name="xin", bufs=4))
        vtmp = ctx.enter_context(tc.tile_pool(name="vtmp", bufs=2))
        etp = ctx.enter_context(tc.tile_pool(name="etp", bufs=4))
        ysb = ctx.enter_context(tc.tile_pool(name="ysb", bufs=3))
        nrm = ctx.enter_context(tc.tile_pool(name="nrm", bufs=4))
        psS = ctx.enter_context(tc.tile_pool(name="psS", bufs=2, space="PSUM"))
        psA = ctx.enter_context(tc.tile_pool(name="psA", bufs=3, space="PSUM"))
        psY = ctx.enter_context(tc.tile_pool(name="psY", bufs=1, space="PSUM"))

        # identity built on DVE (earliest-booting engine); needed by the
        # first transpose
        ident = consts.tile([P, P], DTB, tag="ident")
        make_identity(nc, ident)

        # ---- input DMAs: few, large, contiguous; x first (PE critical path)
        xg = [consts.tile([P, 4, D], DTB, tag=f"xg{sg}", name=f"xg{sg}")
              for sg in range(4)]
        wq_all = consts.tile([P, KC, DG], DTB, tag="wq")
        wkv_all = consts.tile([P, KC, 2 * HD], DTB, tag="wkv")
        wo_all = consts.tile([P, 2, D], DTB, tag="wo")
        bq_all = consts.tile([P, 2, 1], DT, tag="bq")
        bk_sb = consts.tile([HD, 1], DT, tag="bk")
        bv_all = consts.tile([P, 2, 1], DT, tag="bv")

        nc.gpsimd.dma_start(xg[0], xc[:, 0:4, :])
        nc.sync.dma_start(xg[1], xc[:, 4:8, :])
        nc.gpsimd.dma_start(xg[2], xc[:, 8:12, :])
        nc.sync.dma_start(wq_all, wqd[:])
        nc.gpsimd.dma_start(wkv_all, wkvd[:])
        nc.gpsimd.dma_start(bk_sb, bkd[:])
        nc.sync.dma_start(xg[3], xc[:, 12:16, :])
        nc.sync.dma_start(bq_all, bqd[:])
        nc.sync.dma_start(wo_all, wod[:])
        nc.sync.dma_start(bv_all, bvd[:])

        xT = [consts.tile([P, S], DTB, tag=f"xT{dc}", name=f"xT{dc}") for dc in range(KC)]
        qT = [consts.tile([HD, S], DTB, tag=f"qT{h}", name=f"qT{h}") for h in range(GQ)]
        kT = consts.tile([HD, S], DTB, tag="kT")
        vA = consts.tile([P, NKT, HD + 1], DTB, tag="vA")
        oT = [consts.tile([P, S], DTB, tag=f"oT{c}", name=f"oT{c}") for c in range(2)]
        nc.vector.memset(vA[:, :, HD:HD + 1], 1.0)

        # ---- phase 1 + 2 interleaved per 512-seq group ----
        cp_i = 0
        for sg in range(4):
            sl = slice(sg * 512, (sg + 1) * 512)
            for dc in range(KC):
                ps = psA.tile([P, 512], DTB, tag="ps")
                for j in range(4):
                    nc.tensor.transpose(
                        ps[:, j * P:(j + 1) * P],
                        xg[sg][:, j, dc * P:(dc + 1) * P], ident)
                # alternate PSUM->SBUF copies between DVE and ACT
                cp_i += 1
                if (cp_i % 8) < 5:
                    nc.vector.tensor_copy(xT[dc][:, sl], ps)
                else:
                    nc.scalar.activation(xT[dc][:, sl], ps, AF.Identity)
            # Q projection for this seq group
            for mc in range(2):
                ps = psA.tile([P, 512], DT, tag="ps")
                for kc in range(KC):
                    nc.tensor.matmul(
                        ps, wq_all[:, kc, mc * P:(mc + 1) * P], xT[kc][:, sl],
                        start=(kc == 0), stop=(kc == KC - 1))
                for hh in range(2):
                    h = mc * 2 + hh
                    nc.scalar.activation(
                        qT[h][:, sl], ps[hh * HD:(hh + 1) * HD, :], AF.Identity,
                        bias=bq_all[hh * HD:(hh + 1) * HD, mc, :], scale=0.125)
            # KV projection for this seq group
            ps2 = psA.tile([P, 512], DT, tag="ps")
            for kc in range(KC):
                nc.tensor.matmul(
                    ps2, wkv_all[:, kc, :], xT[kc][:, sl],
                    start=(kc == 0), stop=(kc == KC - 1))
            nc.scalar.activation(kT[:, sl], ps2[0:HD, :], AF.Identity, bias=bk_sb)
            vt = vtmp.tile([HD, 512], DTB, tag="vtmp")
            nc.vector.tensor_copy(vt, ps2[HD:2 * HD, :])
            for j in range(4):
                kt = sg * 4 + j
                vps = psA.tile([P, HD], DTB, tag="ps")
                nc.tensor.transpose(vps, vt[:, j * P:(j + 1) * P], ident[0:HD, 0:HD])
                nc.vector.tensor_copy(vA[:, kt, 0:HD], vps)

        # causal 0/1 masks for diagonal tiles, [128, 2 heads, 512]:
        # keep iff q_local >= 128*j + p  <=>  -128j + q - p >= 0
        # (built on gpsimd after its input-DMA issues, ready well before
        # the first diagonal tile of attention)
        dmask = []
        for j in range(4):
            mt = consts.tile([P, 2, 512], DTB, tag=f"dmask{j}", name=f"dmask{j}")
            nc.gpsimd.memset(mt, 1.0)
            nc.gpsimd.affine_select(
                out=mt, in_=mt, pattern=[[0, 2], [1, 512]],
                compare_op=mybir.AluOpType.is_ge, fill=0.0,
                base=-128 * j, channel_multiplier=-1)
            dmask.append(mt)

        # ---- phases 3 + 4: attention with output-projection filler ----
        filler = []      # (st, nb2) output-projection jobs ready to emit
        tl = [0]

        def emit_filler(tail=False):
            st, nb2 = filler.pop(0)
            tl[0] += 1
            # during the drain (attention pools idle) rotate psums through
            # psS as well so jobs pipeline 3-deep instead of serializing on
            # the single psY bank
            if tail and tl[0] % 3:
                yps = psS.tile([P, 512], DT, tag="sd", name=f"yps{tl[0]}")
            else:
                yps = psY.tile([P, 512], DT, tag="psy", name=f"ypsy{tl[0]}")
            for c in range(2):
                nc.tensor.matmul(
                    yps, oT[c][:, st * P:(st + 1) * P],
                    wo_all[:, c, nb2 * 512:(nb2 + 1) * 512],
                    start=(c == 0), stop=(c == 1))
            yt = ysb.tile([P, 512], DT, tag="y")
            if tail and tl[0] % 2:
                nc.scalar.activation(yt, yps, AF.Identity)
            else:
                nc.vector.tensor_copy(yt, yps)
            (nc.sync if (st * 2 + nb2) % 2 else nc.gpsimd).dma_start(
                out_p[st * P:(st + 1) * P, nb2 * 512:(nb2 + 1) * 512], yt)

        for qb in range(NQB):
            qsl = slice(qb * 512, (qb + 1) * 512)
            nkt = 4 * (qb + 1)
            n_iters = 2 * nkt
            gap = max(1, n_iters // len(filler)) if filler else 0
            it = 0
            for pp in range(2):          # head-pair pass
                accs = [
                    psA.tile([HD + 1, 512], DT, tag="ps", name=f"acc{qb}_{pp}_{i}")
                    for i in range(2)
                ]
                # AV matmuls lag the score/exp stream by 2 key-tiles so the
                # PE never head-of-line blocks on exp+mask of the current
                # tile, which keeps ACT continuously fed.
                pend = []

                def flush_av():
                    fkt, fet, fc0 = pend.pop(0)
                    for hh in range(2):
                        nc.tensor.matmul(
                            accs[hh][:, fc0:], vA[:, fkt, :], fet[:, hh, fc0:],
                            start=(fkt == 0), stop=(fkt == nkt - 1))

                for kt in range(nkt):
                    sps = psS.tile([P, 2, 512], DT, tag="sd")
                    for hh in range(2):
                        h = pp * 2 + hh
                        nc.tensor.matmul(
                            sps[:, hh, :], kT[:, kt * P:(kt + 1) * P],
                            qT[h][:, qsl], start=True, stop=True)
                    j = kt - 4 * qb
                    # columns < 128*j are fully masked: skip them end-to-end
                    # (exp, mask, AV). kt==0 always has c0==0, so the psum
                    # has_written bits cover the full accumulator width.
                    c0 = max(j, 0) * P
                    et = etp.tile([P, 2, 512], DTB, tag="et")
                    nc.scalar.activation(
                        et[:, :, c0:], sps[:, :, c0:], AF.Exp)
                    if j >= 0:
                        nc.vector.tensor_mul(
                            et[:, :, c0:], et[:, :, c0:], dmask[j][:, :, c0:])
                    pend.append((kt, et, c0))
                    if len(pend) > 2:
                        flush_av()
                    it += 1
                    if filler and gap and it % gap == 0:
                        emit_filler()
                while pend:
                    flush_av()
                # normalize this pass's two heads (off PE critical path)
                for hh in range(2):
                    h = pp * 2 + hh
                    c, r0 = h // 2, (h % 2) * HD
                    oU = nrm.tile([HD, 512], DTB, tag="oU")
                    nc.vector.tensor_copy(oU, accs[hh][0:HD, :])
                    dsb = nrm.tile([1, 512], DT, tag="dsb")
                    nc.vector.tensor_copy(dsb, accs[hh][HD:HD + 1, :])
                    rcp = nrm.tile([1, 512], DT, tag="rcp")
                    # custom-DVE op: SBUF operands only (PSUM input returns
                    # garbage on HW; verified empirically)
                    nc.vector.reciprocal_approx_fast(out=rcp, in_=dsb)
                    rcpb = nrm.tile([1, 512], DTB, tag="rcpb")
                    nc.vector.tensor_copy(rcpb, rcp)
                    rbb = nrm.tile([HD, 512], DTB, tag="rbb")
                    nc.gpsimd.partition_broadcast(rbb, rcpb, channels=HD)
                    nc.vector.tensor_mul(oT[c][r0:r0 + HD, qsl], oU, rbb)
                # v bias (equivalent to adding bv to v since sum(attn)=1)
                nc.vector.tensor_scalar_add(
                    oT[pp][:, qsl], oT[pp][:, qsl], bv_all[:, pp, :])
            while filler:
                emit_filler(tail=True)
            for st in range(4 * qb, 4 * qb + 4):
                filler.append((st, 0))
                filler.append((st, 1))
        while filler:
            emit_filler(tail=True)

    nc.compile()
    return nc


def kernel(x, mask, wq, bq, wk, bk, wv, bv, wo, bo):
    x = np.asarray(x, dtype=np.float32)
    wq = np.asarray(wq, dtype=np.float32)
    wk = np.asarray(wk, dtype=np.float32)
    wv = np.asarray(wv, dtype=np.float32)
    wo = np.asarray(wo, dtype=np.float32)
    bq = np.asarray(bq, dtype=np.float32)
    bk = np.asarray(bk, dtype=np.float32)
    bv = np.asarray(bv, dtype=np.float32)
    bo = np.asarray(bo, dtype=np.float32)

    xb = x.astype(BF16)
    wqb = wq.astype(BF16)
    wkb = wk.astype(BF16)
    wvb = wv.astype(BF16)
    wob = wo.astype(BF16)

    def pack(a, groups):
        # [groups*128, F] -> [128, groups, F]
        return np.ascontiguousarray(
            a.reshape(groups, P, -1).transpose(1, 0, 2))

    in_maps = []
    for c in range(N_CORES):
        b, g = c // 4, c % 4
        sq = slice(g * DG, (g + 1) * DG)
        sk = slice(g * HD, (g + 1) * HD)
        wkv = np.concatenate([wkb[:, sk], wvb[:, sk]], axis=1)
        in_maps.append({
            "xc": pack(xb[b], NKT),
            "wqd": pack(wqb[:, sq], KC),
            "wkvd": pack(wkv, KC),
            "wod": pack(wob[sq, :], 2),
            "bqd": pack((bq[sq] * 0.125).reshape(DG, 1), 2),
            "bkd": np.ascontiguousarray(bk[sk].reshape(HD, 1)),
            "bvd": pack(np.tile(bv[sk], GQ).reshape(DG, 1), 2),
        })

    results = _run(in_maps)

    out = np.empty((B, S, D), dtype=np.float32)
    for b in range(B):
        acc = results[b * 4 + 0]["out_p"].astype(np.float64)
        for g in range(1, 4):
            acc += results[b * 4 + g]["out_p"]
        out[b] = (acc + bo).astype(np.float32)
    return out


def _get_runner():
    """Build (once) a jitted shard_map callable executing the compiled
    kernel on 8 cores. Adapted from concourse.bass2jax.run_bass_via_pjrt,
    minus output-buffer donation so the callable is re-invokable for
    timing."""
    if "runner" in _CACHE:
        return _CACHE["runner"]
    import jax
    from jax.experimental.shard_map import shard_map
    from jax.sharding import Mesh, PartitionSpec
    from concourse import bass2jax
    from concourse.bass2jax import _bass_exec_p, install_neuronx_cc_hook

    install_neuronx_cc_hook()
    nc = build_nc()
    _CACHE["nc"] = nc
    partition_name = (
        nc.partition_id_tensor.name if nc.partition_id_tensor else None
    )

    in_names, out_names, out_avals, zero_outs = [], [], [], []
    for alloc in nc.m.functions[0].allocations:
        if not isinstance(alloc, mybir.MemoryLocationSet):
            continue
        name = alloc.memorylocations[0].name
        if alloc.kind == "ExternalInput":
            if name != partition_name:
                in_names.append(name)
        elif alloc.kind == "ExternalOutput":
            out_names.append(name)
            shape = tuple(alloc.tensor_shape)
            dtype = mybir.dt.np(alloc.dtype)
            out_avals.append(jax.core.ShapedArray(shape, dtype))
            zero_outs.append(np.zeros(shape, dtype))
    n_params = len(in_names)
    all_names = in_names + out_names
    if partition_name is not None:
        all_names = all_names + [partition_name]

    def _body(*args):
        operands = list(args)
        if partition_name is not None:
            operands.append(bass2jax.partition_id_tensor())
        outs = _bass_exec_p.bind(
            *operands,
            out_avals=tuple(out_avals),
            in_names=tuple(all_names),
            out_names=tuple(out_names),
            lowering_input_output_aliases=(),
            sim_require_finite=True,
            sim_require_nnan=True,
            nc=nc,
        )
        return tuple(outs)

    devices = jax.devices()[:N_CORES]
    mesh = Mesh(np.asarray(devices), ("core",))
    n_all = n_params + len(out_names)
    sharded = jax.jit(
        shard_map(
            _body,
            mesh=mesh,
            in_specs=(PartitionSpec("core"),) * n_all,
            out_specs=(PartitionSpec("core"),) * len(out_names),
            check_rep=False,
        ),
        keep_unused=True,
    )
    runner = {
        "sharded": sharded,
        "in_names": in_names,
        "out_names": out_names,
        "out_avals": out_avals,
        "zero_outs": zero_outs,
        "mesh": mesh,
    }
    _CACHE["runner"] = runner
    return runner


def _run(in_maps):
    r = _get_runner()
    concat_in = [
        np.concatenate([np.asarray(in_maps[c][n]) for c in range(N_CORES)], axis=0)
        for n in r["in_names"]
    ]
    concat_zeros = [
        np.zeros((N_CORES * z.shape[0], *z.shape[1:]), z.dtype)
        for z in r["zero_outs"]
    ]
    out_arrs = r["sharded"](*concat_in, *concat_zeros)
    _CACHE["last_args"] = (concat_in, concat_zeros)
    return [
        {
            n: np.asarray(out_arrs[i]).reshape(
                N_CORES, *r["out_avals"][i].shape
            )[c]
            for i, n in enumerate(r["out_names"])
        }
        for c in range(N_CORES)
    ]


def bench(iters=10):
    """Re-execute the last-run kernel with device-resident inputs and
    return per-call wall times (s). Outputs stay on device."""
    import time as _time
    import jax
    from jax.sharding import NamedSharding, PartitionSpec

    r = _CACHE["runner"]
    concat_in, concat_zeros = _CACHE["last_args"]
    sh = NamedSharding(r["mesh"], PartitionSpec("core"))
    dev_args = [jax.device_put(a, sh) for a in (*concat_in, *concat_zeros)]
    for a in dev_args:
        a.block_until_ready()
    times = []
    for _ in range(iters):
        t0 = _time.perf_counter()
        outs = r["sharded"](*dev_args)
        for o in outs:
            o.block_until_ready()
        times.append(_time.perf_counter() - t0)
    return times


def profile_exec(prof_dir=None):
    """Capture an NTFF profile (neuron-profile) of one device-resident
    re-execution; return (max_exec_time_ns, per_core_ns, trace_paths)."""
    import contextlib
    import ctypes
    import tempfile
    import jax
    from jax.sharding import NamedSharding, PartitionSpec

    r = _CACHE["runner"]
    concat_in, concat_zeros = _CACHE["last_args"]
    sh = NamedSharding(r["mesh"], PartitionSpec("core"))
    dev_args = [jax.device_put(a, sh) for a in (*concat_in, *concat_zeros)]
    for a in dev_args:
        a.block_until_ready()
    # warm run
    outs = r["sharded"](*dev_args)
    for o in outs:
        o.block_until_ready()

    lib = ctypes.CDLL("/opt/axon/libaxon_pjrt.so")
    lib.axon_start_nrt_profile.argtypes = [
        ctypes.POINTER(ctypes.c_int64), ctypes.c_size_t]
    lib.axon_start_nrt_profile.restype = ctypes.c_int64
    lib.axon_stop_nrt_profile.argtypes = [ctypes.c_char_p]
    lib.axon_stop_nrt_profile.restype = ctypes.c_int64

    if prof_dir is None:
        prof_dir = tempfile.mkdtemp(prefix="ntffprof_")
    ids = (ctypes.c_int64 * N_CORES)(*range(N_CORES))
    rc = lib.axon_start_nrt_profile(ids, N_CORES)
    if rc != 0:
        raise RuntimeError(f"axon_start_nrt_profile rc={rc}")
    try:
        outs = r["sharded"](*dev_args)
        for o in outs:
            o.block_until_ready()
    finally:
        n = lib.axon_stop_nrt_profile(str(prof_dir).encode())
        if n <= 0:
            raise RuntimeError(f"axon_stop_nrt_profile wrote {n} files")

    import gauge.profiler
    profile = gauge.profiler.Profile(
        profile_path=gauge.profiler.FishPath(prof_dir),
        kernel_dev_mode=True,
        profile_on_exit=False,
        bass_kernel=_CACHE["nc"].m,
        offline_processing=True,
        fname="*_body*",
    )
    ntffs = profile.find_ntffs()
    model_indices = tuple(sorted(set(n.model_index for n in ntffs)))
    results = profile.to_perfetto(model_index=model_indices)
    per_core = {mi: res.exec_time_ns for mi, res in zip(model_indices, results)}
    traces = {mi: res.trace_path for mi, res in zip(model_indices, results)}
    return max(per_core.values()), per_core, traces, prof_dir


# revision 17
# speedup vs baseline: 1.2069x; 1.0416x over previous
"""GQA attention kernel for 8 Trainium2 NeuronCores.

Problem: B=2, S=2048, D=1024, 16 Q heads / 4 KV heads (GQA), causal,
y = softmax((x@wq+bq)(x@wk+bk)^T / 8, causal) @ (x@wv+bv) @ wo + bo

Sharding: core c -> (batch b = c//4, kv-group g = c%4). Each core computes
its batch's attention for 4 Q heads (= 1 KV head) and the partial output
projection through wo[g*256:(g+1)*256, :]. Host sums the 4 partials per
batch and adds bo.

Per-core pipeline (engineered for a continuously-busy PE so the HAM clock
gate stays at 8/8 = 2.4 GHz):
  phase 1+2 (interleaved per 512-seq group): x tiles DMA'd up front;
    PE transposes x -> xT; Q/KV projection chains run per group as soon
    as its xT columns land, overlapping the remaining x DMA.
  phase 3 attention, per (q-block, head-pair, key-tile):
    2 score matmuls -> one [128,2,512] fp32 PSUM tile; ONE wide exp on
    ACT (causally trimmed columns); prebuilt causal masks applied by DVE
    bf16 multiplies; 2 AV matmuls accumulate [65,512] (row 64 = softmax
    denominator via the ones column of vA).
    Normalization runs off the critical path: DVE evicts acc + computes
    reciprocal_approx_fast; GpSimd broadcasts 1/den across partitions;
    DVE multiplies into oT.
  phase 4 (output projection) is interleaved into the NEXT q-block's
    attention stream as PE filler; results DMA out per 128-row tile.
"""

import os
import sys
from contextlib import ExitStack

import numpy as np
import ml_dtypes

if "/opt/trn_rl_repo" not in sys.path:
    sys.path.insert(0, "/opt/trn_rl_repo")

import concourse.bass as bass
import concourse.tile as tile
from concourse import bacc, mybir
from concourse.masks import make_identity

B, S, D = 2, 2048, 1024
H, KVH, HD = 16, 4, 64
GQ = H // KVH        # 4 q heads per core
DG = GQ * HD         # 256 q dims per core
P = 128
KC = D // P          # 8 contraction chunks over D
NKT = S // P         # 16 key tiles
NQB = S // 512       # 4 query blocks
N_CORES = 8

DT = mybir.dt.float32
DTB = mybir.dt.bfloat16
AF = mybir.ActivationFunctionType
BF16 = ml_dtypes.bfloat16

_CACHE = {}


def build_nc():
    nc = bacc.Bacc(
        "TRN2",
        target_bir_lowering=False,
        debug=False,
        enable_asserts=False,
        num_devices=N_CORES,
    )
    xc = nc.dram_tensor("xc", [S, D], DTB, kind="ExternalInput").ap()
    wqd = nc.dram_tensor("wqd", [D, DG], DTB, kind="ExternalInput").ap()
    wkd = nc.dram_tensor("wkd", [D, HD], DTB, kind="ExternalInput").ap()
    wvd = nc.dram_tensor("wvd", [D, HD], DTB, kind="ExternalInput").ap()
    wod = nc.dram_tensor("wod", [DG, D], DTB, kind="ExternalInput").ap()
    bqd = nc.dram_tensor("bqd", [DG, 1], DT, kind="ExternalInput").ap()
    bkd = nc.dram_tensor("bkd", [HD, 1], DT, kind="ExternalInput").ap()
    bvd = nc.dram_tensor("bvd", [DG, 1], DT, kind="ExternalInput").ap()
    out_p = nc.dram_tensor("out_p", [S, D], DT, kind="ExternalOutput").ap()

    with tile.TileContext(nc) as tc, ExitStack() as ctx:
        consts = ctx.enter_context(tc.tile_pool(name="consts", bufs=1))
        xin = ctx.enter_context(tc.tile_pool(name="xin", bufs=4))
        vtmp = ctx.enter_context(tc.tile_pool(name="vtmp", bufs=2))
        etp = ctx.enter_context(tc.tile_pool(name="etp", bufs=4))
        ysb = ctx.enter_context(tc.tile_pool(name="ysb", bufs=3))
        nrm = ctx.enter_context(tc.tile_pool(name="nrm", bufs=4))
        psS = ctx.enter_context(tc.tile_pool(name="psS", bufs=2, space="PSUM"))
        psA = ctx.enter_context(tc.tile_pool(name="psA", bufs=3, space="PSUM"))
        psY = ctx.enter_context(tc.tile_pool(name="psY", bufs=1, space="PSUM"))

        # identity built on DVE (earliest-booting engine); needed by the
        # first transpose
        ident = consts.tile([P, P], DTB, tag="ident")
        make_identity(nc, ident)

        # ---- input DMAs: few, large, strided; x first (PE critical path).
        # gpsimd boots ~2us before sync; it carries the first x chunk.
        xg = [consts.tile([P, 4, D], DTB, tag=f"xg{sg}", name=f"xg{sg}")
              for sg in range(4)]
        wq_all = consts.tile([P, KC, DG], DTB, tag="wq")
        wkv_all = consts.tile([P, KC, 2 * HD], DTB, tag="wkv")
        wo_sb = [consts.tile([P, D], DTB, tag=f"wo{c}", name=f"wo{c}") for c in range(2)]
        bq_all = consts.tile([P, 2, 1], DT, tag="bq")
        bk_sb = consts.tile([HD, 1], DT, tag="bk")
        bv_all = consts.tile([P, 2, 1], DT, tag="bv")

        def xsrc(sg):
            # [128, 4, D] view of x rows sg*512 .. (sg+1)*512
            return xc[sg * 512:(sg + 1) * 512, :].rearrange(
                "(j p) d -> p j d", p=P)

        nc.gpsimd.dma_start(xg[0], xsrc(0))
        nc.sync.dma_start(xg[1], xsrc(1))
        nc.sync.dma_start(wq_all, wqd.rearrange("(kc p) d -> p kc d", p=P))
        nc.sync.dma_start(bq_all, bqd.rearrange("(mc p) o -> p mc o", p=P))
        nc.gpsimd.dma_start(wkv_all[:, :, 0:HD], wkd.rearrange("(kc p) d -> p kc d", p=P))
        nc.gpsimd.dma_start(wkv_all[:, :, HD:2 * HD], wvd.rearrange("(kc p) d -> p kc d", p=P))
        nc.gpsimd.dma_start(bk_sb, bkd[:, :])
        nc.gpsimd.dma_start(xg[2], xsrc(2))
        nc.sync.dma_start(xg[3], xsrc(3))
        for c in range(2):
            nc.sync.dma_start(wo_sb[c], wod[c * P:(c + 1) * P, :])
        nc.sync.dma_start(bv_all, bvd.rearrange("(c p) o -> p c o", p=P))

        xT = [consts.tile([P, S], DTB, tag=f"xT{dc}", name=f"xT{dc}") for dc in range(KC)]
        qT = [consts.tile([HD, S], DTB, tag=f"qT{h}", name=f"qT{h}") for h in range(GQ)]
        kT = consts.tile([HD, S], DTB, tag="kT")
        vA = consts.tile([P, NKT, HD + 1], DTB, tag="vA")
        oT = [consts.tile([P, S], DTB, tag=f"oT{c}", name=f"oT{c}") for c in range(2)]
        nc.vector.memset(vA[:, :, HD:HD + 1], 1.0)

        # ---- phase 1 + 2 interleaved per 512-seq group ----
        cp_i = 0
        for sg in range(4):
            sl = slice(sg * 512, (sg + 1) * 512)
            for dc in range(KC):
                ps = psA.tile([P, 512], DTB, tag="ps")
                for j in range(4):
                    nc.tensor.transpose(
                        ps[:, j * P:(j + 1) * P],
                        xg[sg][:, j, dc * P:(dc + 1) * P], ident)
                # alternate PSUM->SBUF copies between DVE and ACT
                cp_i += 1
                if (cp_i % 8) < 5:
                    nc.vector.tensor_copy(xT[dc][:, sl], ps)
                else:
                    nc.scalar.activation(xT[dc][:, sl], ps, AF.Identity)
            # Q projection for this seq group
            for mc in range(2):
                ps = psA.tile([P, 512], DT, tag="ps")
                for kc in range(KC):
                    nc.tensor.matmul(
                        ps, wq_all[:, kc, mc * P:(mc + 1) * P], xT[kc][:, sl],
                        start=(kc == 0), stop=(kc == KC - 1))
                for hh in range(2):
                    h = mc * 2 + hh
                    nc.scalar.activation(
                        qT[h][:, sl], ps[hh * HD:(hh + 1) * HD, :], AF.Identity,
                        bias=bq_all[hh * HD:(hh + 1) * HD, mc, :], scale=0.125)
            # KV projection for this seq group
            ps2 = psA.tile([P, 512], DT, tag="ps")
            for kc in range(KC):
                nc.tensor.matmul(
                    ps2, wkv_all[:, kc, :], xT[kc][:, sl],
                    start=(kc == 0), stop=(kc == KC - 1))
            nc.scalar.activation(kT[:, sl], ps2[0:HD, :], AF.Identity, bias=bk_sb)
            vt = vtmp.tile([HD, 512], DTB, tag="vtmp")
            nc.vector.tensor_copy(vt, ps2[HD:2 * HD, :])
            for j in range(4):
                kt = sg * 4 + j
                vps = psA.tile([P, HD], DTB, tag="ps")
                nc.tensor.transpose(vps, vt[:, j * P:(j + 1) * P], ident[0:HD, 0:HD])
                nc.vector.tensor_copy(vA[:, kt, 0:HD], vps)

        # causal 0/1 masks for diagonal tiles, [128, 2 heads, 512]:
        # keep iff q_local >= 128*j + p  <=>  -128j + q - p >= 0
        # (built on gpsimd after its input-DMA issues, ready well before
        # the first diagonal tile of attention)
        dmask = []
        for j in range(4):
            mt = consts.tile([P, 2, 512], DTB, tag=f"dmask{j}", name=f"dmask{j}")
            nc.gpsimd.memset(mt, 1.0)
            nc.gpsimd.affine_select(
                out=mt, in_=mt, pattern=[[0, 2], [1, 512]],
                compare_op=mybir.AluOpType.is_ge, fill=0.0,
                base=-128 * j, channel_multiplier=-1)
            dmask.append(mt)

        # ---- phases 3 + 4: attention with output-projection filler ----
        filler = []      # (st, nb2) output-projection jobs ready to emit
        tl = [0]

        def emit_filler(tail=False):
            st, nb2 = filler.pop(0)
            tl[0] += 1
            # during the drain (attention pools idle) rotate psums through
            # psS as well so jobs pipeline 3-deep instead of serializing on
            # the single psY bank
            if tail and tl[0] % 3:
                yps = psS.tile([P, 512], DT, tag="sd", name=f"yps{tl[0]}")
            else:
                yps = psY.tile([P, 512], DT, tag="psy", name=f"ypsy{tl[0]}")
            for c in range(2):
                nc.tensor.matmul(
                    yps, oT[c][:, st * P:(st + 1) * P],
                    wo_sb[c][:, nb2 * 512:(nb2 + 1) * 512],
                    start=(c == 0), stop=(c == 1))
            yt = ysb.tile([P, 512], DT, tag="y")
            if tail and tl[0] % 2:
                nc.scalar.activation(yt, yps, AF.Identity)
            else:
                nc.vector.tensor_copy(yt, yps)
            (nc.sync if (st * 2 + nb2) % 2 else nc.gpsimd).dma_start(
                out_p[st * P:(st + 1) * P, nb2 * 512:(nb2 + 1) * 512], yt)

        for qb in range(NQB):
            qsl = slice(qb * 512, (qb + 1) * 512)
            nkt = 4 * (qb + 1)
            n_iters = 2 * nkt
            gap = max(1, n_iters // len(filler)) if filler else 0
            it = 0
            for pp in range(2):          # head-pair pass
                accs = [
                    psA.tile([HD + 1, 512], DT, tag="ps", name=f"acc{qb}_{pp}_{i}")
                    for i in range(2)
                ]
                for kt in range(nkt):
                    sps = psS.tile([P, 2, 512], DT, tag="sd")
                    for hh in range(2):
                        h = pp * 2 + hh
                        nc.tensor.matmul(
                            sps[:, hh, :], kT[:, kt * P:(kt + 1) * P],
                            qT[h][:, qsl], start=True, stop=True)
                    j = kt - 4 * qb
                    # columns < 128*j are fully masked: skip them end-to-end
                    # (exp, mask, AV). kt==0 always has c0==0, so the psum
                    # has_written bits cover the full accumulator width.
                    c0 = max(j, 0) * P
                    et = etp.tile([P, 2, 512], DTB, tag="et")
                    nc.scalar.activation(
                        et[:, :, c0:], sps[:, :, c0:], AF.Exp)
                    if j >= 0:
                        nc.vector.tensor_mul(
                            et[:, :, c0:], et[:, :, c0:], dmask[j][:, :, c0:])
                    for hh in range(2):
                        nc.tensor.matmul(
                            accs[hh][:, c0:], vA[:, kt, :], et[:, hh, c0:],
                            start=(kt == 0), stop=(kt == nkt - 1))
                    it += 1
                    if filler and gap and it % gap == 0:
                        emit_filler()
                # normalize this pass's two heads (off PE critical path)
                for hh in range(2):
                    h = pp * 2 + hh
                    c, r0 = h // 2, (h % 2) * HD
                    oU = nrm.tile([HD, 512], DTB, tag="oU")
                    nc.vector.tensor_copy(oU, accs[hh][0:HD, :])
                    dsb = nrm.tile([1, 512], DT, tag="dsb")
                    nc.vector.tensor_copy(dsb, accs[hh][HD:HD + 1, :])
                    rcp = nrm.tile([1, 512], DT, tag="rcp")
                    # custom-DVE op: SBUF operands only (PSUM input returns
                    # garbage on HW; verified empirically)
                    nc.vector.reciprocal_approx_fast(out=rcp, in_=dsb)
                    rcpb = nrm.tile([1, 512], DTB, tag="rcpb")
                    nc.vector.tensor_copy(rcpb, rcp)
                    rbb = nrm.tile([HD, 512], DTB, tag="rbb")
                    nc.gpsimd.partition_broadcast(rbb, rcpb, channels=HD)
                    nc.vector.tensor_mul(oT[c][r0:r0 + HD, qsl], oU, rbb)
                # v bias (equivalent to adding bv to v since sum(attn)=1)
                nc.vector.tensor_scalar_add(
                    oT[pp][:, qsl], oT[pp][:, qsl], bv_all[:, pp, :])
            while filler:
                emit_filler(tail=True)
            for st in range(4 * qb, 4 * qb + 4):
                filler.append((st, 0))
                filler.append((st, 1))
        while filler:
            emit_filler(tail=True)

    nc.compile()
    return nc


def kernel(x, mask, wq, bq, wk, bk, wv, bv, wo, bo):
    x = np.asarray(x, dtype=np.float32)
    wq = np.asarray(wq, dtype=np.float32)
    wk = np.asarray(wk, dtype=np.float32)
    wv = np.asarray(wv, dtype=np.float32)
    wo = np.asarray(wo, dtype=np.float32)
    bq = np.asarray(bq, dtype=np.float32)
    bk = np.asarray(bk, dtype=np.float32)
    bv = np.asarray(bv, dtype=np.float32)
    bo = np.asarray(bo, dtype=np.float32)

    xb = x.astype(BF16)
    wqb = wq.astype(BF16)
    wkb = wk.astype(BF16)
    wvb = wv.astype(BF16)
    wob = wo.astype(BF16)

    in_maps = []
    for c in range(N_CORES):
        b, g = c // 4, c % 4
        sq = slice(g * DG, (g + 1) * DG)
        sk = slice(g * HD, (g + 1) * HD)
        in_maps.append({
            "xc": np.ascontiguousarray(xb[b]),
            "wqd": np.ascontiguousarray(wqb[:, sq]),
            "wkd": np.ascontiguousarray(wkb[:, sk]),
            "wvd": np.ascontiguousarray(wvb[:, sk]),
            "wod": np.ascontiguousarray(wob[sq, :]),
            "bqd": np.ascontiguousarray((bq[sq] * 0.125).reshape(DG, 1)),
            "bkd": np.ascontiguousarray(bk[sk].reshape(HD, 1)),
            "bvd": np.ascontiguousarray(np.tile(bv[sk], GQ).reshape(DG, 1)),
        })

    results = _run(in_maps)

    out = np.empty((B, S, D), dtype=np.float32)
    for b in range(B):
        acc = results[b * 4 + 0]["out_p"].astype(np.float64)
        for g in range(1, 4):
            acc += results[b * 4 + g]["out_p"]
        out[b] = (acc + bo).astype(np.float32)
    return out


def _get_runner():
    """Build (once) a jitted shard_map callable executing the compiled
    kernel on 8 cores. Adapted from concourse.bass2jax.run_bass_via_pjrt,
    minus output-buffer donation so the callable is re-invokable for
    timing."""
    if "runner" in _CACHE:
        return _CACHE["runner"]
    import jax
    from jax.experimental.shard_map import shard_map
    from jax.sharding import Mesh, PartitionSpec
    from concourse import bass2jax
    from concourse.bass2jax import _bass_exec_p, install_neuronx_cc_hook

    install_neuronx_cc_hook()
    nc = build_nc()
    _CACHE["nc"] = nc
    partition_name = (
        nc.partition_id_tensor.name if nc.partition_id_tensor else None
    )

    in_names, out_names, out_avals, zero_outs = [], [], [], []
    for alloc in nc.m.functions[0].allocations:
        if not isinstance(alloc, mybir.MemoryLocationSet):
            continue
        name = alloc.memorylocations[0].name
        if alloc.kind == "ExternalInput":
            if name != partition_name:
                in_names.append(name)
        elif alloc.kind == "ExternalOutput":
            out_names.append(name)
            shape = tuple(alloc.tensor_shape)
            dtype = mybir.dt.np(alloc.dtype)
            out_avals.append(jax.core.ShapedArray(shape, dtype))
            zero_outs.append(np.zeros(shape, dtype))
    n_params = len(in_names)
    all_names = in_names + out_names
    if partition_name is not None:
        all_names = all_names + [partition_name]

    def _body(*args):
        operands = list(args)
        if partition_name is not None:
            operands.append(bass2jax.partition_id_tensor())
        outs = _bass_exec_p.bind(
            *operands,
            out_avals=tuple(out_avals),
            in_names=tuple(all_names),
            out_names=tuple(out_names),
            lowering_input_output_aliases=(),
            sim_require_finite=True,
            sim_require_nnan=True,
            nc=nc,
        )
        return tuple(outs)

    devices = jax.devices()[:N_CORES]
    mesh = Mesh(np.asarray(devices), ("core",))
    n_all = n_params + len(out_names)
    sharded = jax.jit(
        shard_map(
            _body,
            mesh=mesh,
            in_specs=(PartitionSpec("core"),) * n_all,
            out_specs=(PartitionSpec("core"),) * len(out_names),
            check_rep=False,
        ),
        keep_unused=True,
    )
    runner = {
        "sharded": sharded,
        "in_names": in_names,
        "out_names": out_names,
        "out_avals": out_avals,
        "zero_outs": zero_outs,
        "mesh": mesh,
    }
    _CACHE["runner"] = runner
    return runner


def _run(in_maps):
    r = _get_runner()
    concat_in = [
        np.concatenate([np.asarray(in_maps[c][n]) for c in range(N_CORES)], axis=0)
        for n in r["in_names"]
    ]
    concat_zeros = [
        np.zeros((N_CORES * z.shape[0], *z.shape[1:]), z.dtype)
        for z in r["zero_outs"]
    ]
    out_arrs = r["sharded"](*concat_in, *concat_zeros)
    _CACHE["last_args"] = (concat_in, concat_zeros)
    return [
        {
            n: np.asarray(out_arrs[i]).reshape(
                N_CORES, *r["out_avals"][i].shape
            )[c]
            for i, n in enumerate(r["out_names"])
        }
        for c in range(N_CORES)
    ]


def bench(iters=10):
    """Re-execute the last-run kernel with device-resident inputs and
    return per-call wall times (s). Outputs stay on device."""
    import time as _time
    import jax
    from jax.sharding import NamedSharding, PartitionSpec

    r = _CACHE["runner"]
    concat_in, concat_zeros = _CACHE["last_args"]
    sh = NamedSharding(r["mesh"], PartitionSpec("core"))
    dev_args = [jax.device_put(a, sh) for a in (*concat_in, *concat_zeros)]
    for a in dev_args:
        a.block_until_ready()
    times = []
    for _ in range(iters):
        t0 = _time.perf_counter()
        outs = r["sharded"](*dev_args)
        for o in outs:
            o.block_until_ready()
        times.append(_time.perf_counter() - t0)
    return times


def profile_exec(prof_dir=None):
    """Capture an NTFF profile (neuron-profile) of one device-resident
    re-execution; return (max_exec_time_ns, per_core_ns, trace_paths)."""
    import contextlib
    import ctypes
    import tempfile
    import jax
    from jax.sharding import NamedSharding, PartitionSpec

    r = _CACHE["runner"]
    concat_in, concat_zeros = _CACHE["last_args"]
    sh = NamedSharding(r["mesh"], PartitionSpec("core"))
    dev_args = [jax.device_put(a, sh) for a in (*concat_in, *concat_zeros)]
    for a in dev_args:
        a.block_until_ready()
    # warm run
    outs = r["sharded"](*dev_args)
    for o in outs:
        o.block_until_ready()

    lib = ctypes.CDLL("/opt/axon/libaxon_pjrt.so")
    lib.axon_start_nrt_profile.argtypes = [
        ctypes.POINTER(ctypes.c_int64), ctypes.c_size_t]
    lib.axon_start_nrt_profile.restype = ctypes.c_int64
    lib.axon_stop_nrt_profile.argtypes = [ctypes.c_char_p]
    lib.axon_stop_nrt_profile.restype = ctypes.c_int64

    if prof_dir is None:
        prof_dir = tempfile.mkdtemp(prefix="ntffprof_")
    ids = (ctypes.c_int64 * N_CORES)(*range(N_CORES))
    rc = lib.axon_start_nrt_profile(ids, N_CORES)
    if rc != 0:
        raise RuntimeError(f"axon_start_nrt_profile rc={rc}")
    try:
        outs = r["sharded"](*dev_args)
        for o in outs:
            o.block_until_ready()
    finally:
        n = lib.axon_stop_nrt_profile(str(prof_dir).encode())
        if n <= 0:
            raise RuntimeError(f"axon_stop_nrt_profile wrote {n} files")

    import gauge.profiler
    profile = gauge.profiler.Profile(
        profile_path=gauge.profiler.FishPath(prof_dir),
        kernel_dev_mode=True,
        profile_on_exit=False,
        bass_kernel=_CACHE["nc"].m,
        offline_processing=True,
        fname="*_body*",
    )
    ntffs = profile.find_ntffs()
    model_indices = tuple(sorted(set(n.model_index for n in ntffs)))
    results = profile.to_perfetto(model_index=model_indices)
    per_core = {mi: res.exec_time_ns for mi, res in zip(model_indices, results)}
    traces = {mi: res.trace_path for mi, res in zip(model_indices, results)}
    return max(per_core.values()), per_core, traces, prof_dir
